# revision 11
# baseline (speedup 1.0000x reference)
"""GAT layer (gnn_message_passing) Trainium2 Bass kernel, v3.5.

Per-core work (data-parallel over batch B=8, one graph per NeuronCore):
  h   = (x*m) @ W
  e   = leakyrelu(e_l[i] + e_r[j]),  e_l = h@a_l, e_r = h@a_r
  attn= softmax_j(adj&mask ? e : -inf)
  out = LN((attn @ h + x*m) * m) * gamma + beta

Algebra: exp(lrelu(s)) with s = e_l[i]+e_r[j] factorizes as
exp(0.2s)*max(exp(0.8s),1); exp(0.8s) = El8[i]*Er8[j] is rank-1 and the
pure-i factor exp(0.2 e_l[i]) cancels in the softmax.  With
r[j] = m[j]*exp(0.2 e_r[j]) the numerator is
  q[j,i] = adj[i,j] * max(El8[i]*(Er8*r)[j], r[j])
Per-j-block [128,2048] elementwise work: one DVE tensor_scalar (mult+max
against the El8 broadcast row) and one tensor_tensor multiply with the
prefetched {0,1} bf16 adjacency (HWDGE sync queue, streams from t=0).

Matmul orientation: w blocks are STATIONARY, rhs = [ones | h | hs] with
hs = sum_d h, so one stream accumulates out[i,d], the softmax denominator
(col 0) and sum_d of the attention term (col 129).  Output lands row-major
=> no transposes, no PSUM bulk copy, no rowsum bounce, no mean-accum.
Node mask on rows folds into the final LN affine (out = z*rstd*m - mu*rstd*m),
so the x*m residual pass disappears too (raw x is added).
"""

import os
import sys

import numpy as np

if "/opt/trn_rl_repo" not in sys.path:
    sys.path.insert(0, "/opt/trn_rl_repo")

B, N, D = 8, 2048, 128
NB = N // 128
ALPHA = 0.2
EPS = 1e-5
NCORES = 8

# Perf knobs: blocks whose adjacency-mask multiply / residual add run on
# GPSIMD instead of DVE; blocks whose hel copy runs on ScalarE.
GP_TT_BLOCKS = frozenset({3, 7, 11, 15})
GP_ZADD_BLOCKS = frozenset({1, 4, 7, 10, 13})
SC_HEL_BLOCKS = frozenset({1, 3, 5, 7, 9, 11, 13, 15})

_PROG_CACHE = {}
RACE_DETECT = True
SEM_CLEAR_MODE = "skip"
LAST_EXEC_TIME_NS = None
LAST_MEAN_EXEC_TIME_NS = None


def _patch_sem_clear():
    """This environment's walrus rejects EVENT_SEMAPHORE_RANGE_CLEAR
    ("ISA wrong length").  Tile's tail range-clear is unnecessary here
    (runtime resets between executions), so skip it."""
    import concourse.bass as bass

    if getattr(bass.BassEngine, "_gat_sem_clear_patched", False):
        return

    def sem_clear(self, sem):
        return None

    bass.BassEngine.sem_clear = sem_clear
    bass.BassEngine._gat_sem_clear_patched = True


def _split_waits(nc, mybir, max_waits=1):
    """This walrus build allows only one semaphore-wait slot per
    instruction ("Too many sync wait commands").  Hoist extra waits onto
    standalone EventSemaphore carrier instructions placed immediately
    before the offender on the same engine."""
    for f in nc.m.functions:
        for b in f.blocks:
            il = b.instructions
            k = 0
            while k < len(il):
                i = il[k]
                si = i.sync_info
                if si is not None and si.on_wait and len(si.on_wait) > max_waits:
                    waits = list(si.on_wait)
                    extra, keep = waits[:-max_waits], waits[-max_waits:]
                    for j, w in enumerate(extra):
                        ev = mybir.InstEventSemaphore(
                            name=f"{i.name}-wsplit{j}",
                            engine=i.engine,
                            debug=i.debug,
                            sync_info=mybir.SyncInfo(on_wait=[w], on_update=[]),
                        )
                        il.insert(k + j, ev)
                    k += len(extra)
                    i.sync_info = mybir.SyncInfo(
                        on_wait=keep, on_update=list(si.on_update or []))
                k += 1
    return nc


def _parse_blocks(env, default):
    v = os.environ.get(env)
    if v is None:
        return default
    return frozenset(int(x) for x in v.split(",") if x != "")


def _knobs():
    return (_parse_blocks("GAT_TT_G", GP_TT_BLOCKS),
            _parse_blocks("GAT_ZADD_G", GP_ZADD_BLOCKS),
            _parse_blocks("GAT_HEL_S", SC_HEL_BLOCKS))


def _build_program(apply_affine: bool):
    import concourse.bass as bass
    import concourse.tile as tile
    from concourse import mybir
    from concourse.masks import make_identity

    _patch_sem_clear()
    gp_tt, gp_zadd, sc_hel = _knobs()

    fp32 = mybir.dt.float32
    bf16 = mybir.dt.bfloat16
    A = mybir.AluOpType
    F = mybir.ActivationFunctionType

    nc = bass.Bass(use_seq_codegen=True, detect_race_conditions=RACE_DETECT)

    x_in = nc.declare_dram_parameter("x", [N, D], fp32, isOutput=False)
    xt_in = nc.declare_dram_parameter("xt", [D, N], bf16, isOutput=False)
    adj_in = nc.declare_dram_parameter("adj01", [N, N], bf16, isOutput=False)
    maskf = nc.declare_dram_parameter("maskf", [N], fp32, isOutput=False)
    # wfull = [W | W@1 | W@a_l | W@a_r | 1]
    # x@wfull = [h | hs=sum_d h | e_l | e_r | xs=sum_d x]
    wfull_in = nc.declare_dram_parameter("wfull", [D, D + 4], bf16, isOutput=False)
    if apply_affine:
        g_in = nc.declare_dram_parameter("gamma", [D], fp32, isOutput=False)
        b_in = nc.declare_dram_parameter("beta", [D], fp32, isOutput=False)
    out_d = nc.declare_dram_parameter("out", [N, D], fp32, isOutput=True)

    el_dram = nc.dram_tensor("el8_scratch", [N], bf16)

    def bcast(ap, parts=128):
        return bass.AP(tensor=ap.tensor, offset=ap.offset, ap=[[0, parts]] + list(ap.ap))

    with tile.TileContext(nc) as tc:
        with tc.tile_pool(name="persist", bufs=1) as per:
            eps_col = per.tile([128, 1], fp32)
            nc.vector.memset(eps_col, EPS)
            warm = per.tile([128, 1], fp32)
            nc.scalar.activation(out=warm, in_=eps_col, func=F.Exp)

            ident_bf = per.tile([128, 128], bf16)
            make_identity(nc, ident_bf)

            m_col = per.tile([128, NB], fp32)
            nc.sync.dma_start(out=m_col, in_=maskf[:].rearrange("(b p) -> p b", p=128))
            w_full = per.tile([128, D + 4], bf16)
            nc.sync.dma_start(out=w_full, in_=wfull_in[:, :])
            xT_sb = per.tile([128, N], bf16)
            nc.sync.dma_start(out=xT_sb, in_=xt_in[:, :])
            if apply_affine:
                g_bc = per.tile([128, D], fp32)
                nc.sync.dma_start(out=g_bc, in_=bcast(g_in[:]))
                b_bc = per.tile([128, D], fp32)
                nc.sync.dma_start(out=b_bc, in_=bcast(b_in[:]))

            # adjacency prefetch: bf16 {0,1}, one DMA per j-block on the
            # sync HWDGE queue -- streams from t~=0, nothing depends on it
            # until the per-block mask multiply.
            adj_all = per.tile([128, NB, N], bf16)
            for jb in range(NB):
                nc.sync.dma_start(out=adj_all[:, jb, :],
                                  in_=adj_in[jb * 128:(jb + 1) * 128, :])

            x_all = per.tile([128, NB, D], fp32)        # raw x rows (residual)
            # hel layout per block: [ones | h(128) | hs | e_l | e_r | xs]
            hel_all = per.tile([128, NB, D + 6], bf16)
            nc.vector.memset(hel_all[:, :, 0], 1.0)
            el8_bc = per.tile([128, N], bf16)
            er8r_col = per.tile([128, NB], fp32)
            r_col = per.tile([128, NB], fp32)
            z_all = per.tile([128, NB, D], fp32)
            z2s_col = per.tile([128, NB], fp32)
            rm_col = per.tile([128, NB], fp32)
            rc_col = per.tile([128, NB], fp32)
            rstdm_col = per.tile([128, NB], fp32)
            nmr_col = per.tile([128, NB], fp32)

            # ---- prep: h|hs|el|er|xs, exps, el8 broadcast ---------------
            with (
                tc.tile_pool(name="pp", bufs=3) as pp,
                tc.tile_pool(name="pp_ps", bufs=2, space="PSUM") as pp_ps,
            ):
                for ib in range(NB):
                    hr_ps = pp_ps.tile([128, D + 4], fp32, tag="hr")
                    nc.tensor.matmul(hr_ps,
                                     lhsT=xT_sb[:, ib * 128:(ib + 1) * 128],
                                     rhs=w_full, start=True, stop=True)
                    ceng = nc.scalar if ib in sc_hel else nc.vector
                    if ceng is nc.scalar:
                        nc.scalar.activation(out=hel_all[:, ib, 1:D + 5],
                                             in_=hr_ps, func=F.Copy)
                    else:
                        nc.vector.tensor_copy(out=hel_all[:, ib, 1:D + 5],
                                              in_=hr_ps)
                    nc.sync.dma_start(out=x_all[:, ib, :],
                                      in_=x_in[ib * 128:(ib + 1) * 128, :])

                el_v = hel_all[:, :, D + 2]     # [128, NB] strided bf16
                er_v = hel_all[:, :, D + 3]
                el8_col = pp.tile([128, NB], bf16, tag="el8")
                nc.scalar.activation(out=el8_col, in_=el_v, func=F.Exp, scale=0.8)
                er8_col = pp.tile([128, NB], fp32, tag="er8")
                nc.scalar.activation(out=er8_col, in_=er_v, func=F.Exp, scale=0.8)
                er2_col = pp.tile([128, NB], fp32, tag="er2")
                nc.scalar.activation(out=er2_col, in_=er_v, func=F.Exp, scale=0.2)
                nc.vector.tensor_tensor(out=r_col, in0=er2_col, in1=m_col,
                                        op=A.mult)
                nc.vector.tensor_tensor(out=er8r_col, in0=er8_col, in1=r_col,
                                        op=A.mult)

                elT_ps = pp_ps.tile([NB, 128], bf16, tag="elT")
                nc.tensor.transpose(elT_ps, el8_col, ident_bf)
                elT_sb = pp.tile([NB, 128], bf16, tag="elTs")
                nc.vector.tensor_copy(out=elT_sb, in_=elT_ps)
                nc.gpsimd.dma_start(out=el_dram[:].rearrange("(b q) -> b q", q=128),
                                    in_=elT_sb)
                nc.gpsimd.dma_start(out=el8_bc, in_=bcast(el_dram[:]))

            # ---- main: w stationary, rhs=[ones|h|hs] --------------------
            # out_all[:, ib, 0]=rowsum, 1:129=(num @ h)[i,d], 129=S1[i]
            with (
                tc.tile_pool(name="mm_ps", bufs=1, space="PSUM") as mm_ps_pool,
                tc.tile_pool(name="wp", bufs=4) as wp,
            ):
                out_all = mm_ps_pool.tile([128, NB, 256], fp32)
                for jb in range(NB):
                    w_t = wp.tile([128, N], bf16, tag="w")
                    nc.vector.tensor_scalar(
                        out=w_t, in0=el8_bc,
                        scalar1=er8r_col[:, jb:jb + 1],
                        scalar2=r_col[:, jb:jb + 1],
                        op0=A.mult, op1=A.max)
                    eng = nc.gpsimd if jb in gp_tt else nc.vector
                    eng.tensor_tensor(out=w_t, in0=w_t,
                                      in1=adj_all[:, jb, :], op=A.mult)
                    st, sp = (jb == 0), (jb == NB - 1)
                    for ib in range(NB):
                        nc.tensor.matmul(
                            out_all[:, ib, 0:130],
                            lhsT=w_t[:, ib * 128:(ib + 1) * 128],
                            rhs=hel_all[:, jb, 0:130],
                            start=st, stop=sp)

                # ---- epilogue -------------------------------------------
                with tc.tile_pool(name="ep", bufs=4) as ep:
                    rs_sb = ep.tile([128, NB], fp32, tag="rs")
                    nc.vector.tensor_copy(out=rs_sb, in_=out_all[:, :, 0])
                    nc.vector.reciprocal(out=rc_col, in_=rs_sb)
                    nc.vector.tensor_tensor(out=rm_col, in0=rc_col, in1=m_col,
                                            op=A.mult)

                    for ib in range(NB):
                        z1 = ep.tile([128, 128], fp32, tag="z1")
                        nc.scalar.activation(out=z1, in_=out_all[:, ib, 1:129],
                                             func=F.Copy,
                                             scale=rc_col[:, ib:ib + 1])
                        zeng = nc.gpsimd if ib in gp_zadd else nc.vector
                        zeng.tensor_tensor(
                            out=z_all[:, ib, :], in0=z1, in1=x_all[:, ib, :],
                            op=A.add)
                        sq = ep.tile([128, 128], fp32, tag="sq")
                        nc.scalar.activation(out=sq, in_=z_all[:, ib, :],
                                             func=F.Square,
                                             accum_out=z2s_col[:, ib:ib + 1])

                    # S1 = col 129; sum_d z = rc*S1 + xs ; mu = that / D
                    s1_sb = ep.tile([128, NB], fp32, tag="s1")
                    nc.vector.tensor_copy(out=s1_sb, in_=out_all[:, :, 129])
                    zsum = ep.tile([128, NB], fp32, tag="zsum")
                    nc.vector.tensor_tensor(out=zsum, in0=s1_sb, in1=rc_col,
                                            op=A.mult)
                    xs_v = hel_all[:, :, D + 4]
                    nc.vector.tensor_tensor(out=zsum, in0=zsum, in1=xs_v,
                                            op=A.add)
                    mu_col = ep.tile([128, NB], fp32, tag="mu")
                    nc.vector.tensor_scalar(out=mu_col, in0=zsum,
                                            scalar1=1.0 / D, scalar2=None,
                                            op0=A.mult)
                    mu2_col = ep.tile([128, NB], fp32, tag="mu2")
                    nc.vector.tensor_tensor(out=mu2_col, in0=mu_col, in1=mu_col,
                                            op=A.mult)
                    var_col = ep.tile([128, NB], fp32, tag="var")
                    nc.vector.tensor_scalar(out=var_col, in0=z2s_col,
                                            scalar1=1.0 / D, scalar2=None,
                                            op0=A.mult)
                    nc.vector.tensor_tensor(out=var_col, in0=var_col,
                                            in1=mu2_col, op=A.subtract)
                    lnv_col = ep.tile([128, NB], fp32, tag="lnv")
                    nc.scalar.activation(out=lnv_col, in_=var_col, func=F.Ln,
                                         bias=eps_col, scale=1.0)
                    rstd_col = ep.tile([128, NB], fp32, tag="rstd")
                    nc.scalar.activation(out=rstd_col, in_=lnv_col, func=F.Exp,
                                         scale=-0.5)
                    # fold node mask into the affine: scale=rstd*m, bias=-mu*scale
                    nc.vector.tensor_tensor(out=rstdm_col, in0=rstd_col,
                                            in1=m_col, op=A.mult)
                    nc.vector.tensor_tensor(out=nmr_col, in0=mu_col,
                                            in1=rstdm_col, op=A.mult)
                    nc.vector.tensor_scalar(out=nmr_col, in0=nmr_col,
                                            scalar1=-1.0, scalar2=None,
                                            op0=A.mult)

                    for ib in range(NB):
                        o_t = ep.tile([128, D], fp32, tag="o")
                        nc.scalar.activation(out=o_t, in_=z_all[:, ib, :],
                                             func=F.Identity,
                                             bias=nmr_col[:, ib:ib + 1],
                                             scale=rstdm_col[:, ib:ib + 1])
                        if apply_affine:
                            nc.vector.tensor_tensor(out=o_t, in0=o_t, in1=g_bc,
                                                    op=A.mult)
                            nc.vector.tensor_tensor(out=o_t, in0=o_t, in1=b_bc,
                                                    op=A.add)
                        nc.gpsimd.dma_start(out=out_d[ib * 128:(ib + 1) * 128, :],
                                            in_=o_t)
    return _split_waits(nc, mybir)


def _get_program(apply_affine: bool):
    key = (apply_affine, _knobs())
    if key not in _PROG_CACHE:
        _PROG_CACHE[key] = _build_program(apply_affine)
    return _PROG_CACHE[key]


def _prep_inputs(x, adj_bool, node_mask, W, a_l, a_r, gamma, beta, apply_affine):
    import ml_dtypes

    bf16 = ml_dtypes.bfloat16
    x = np.asarray(x, dtype=np.float32)
    adj_bool = np.asarray(adj_bool)
    node_mask = np.asarray(node_mask)
    W32 = np.asarray(W, dtype=np.float32)
    wal = W32 @ np.asarray(a_l, np.float32)
    war = W32 @ np.asarray(a_r, np.float32)
    ws = W32.sum(axis=1)
    ones = np.ones((D, 1), np.float32)
    wfull = np.concatenate([W32, ws[:, None], wal[:, None], war[:, None], ones],
                           axis=1)
    wfull_bf = np.ascontiguousarray(wfull.astype(bf16))
    in_maps = []
    for b in range(NCORES):
        adj01 = adj_bool[b].T.astype(np.float32).astype(bf16)
        m = {
            "x": np.ascontiguousarray(x[b]),
            "xt": np.ascontiguousarray(x[b].T.astype(bf16)),
            "adj01": np.ascontiguousarray(adj01),
            "maskf": np.ascontiguousarray(node_mask[b].astype(np.float32)),
            "wfull": wfull_bf,
        }
        if apply_affine:
            m["gamma"] = np.ascontiguousarray(np.asarray(gamma, np.float32))
            m["beta"] = np.ascontiguousarray(np.asarray(beta, np.float32))
        in_maps.append(m)
    return in_maps


def kernel(x, adj_bool, node_mask, W, a_l, a_r, gamma, beta):
    global LAST_EXEC_TIME_NS, LAST_MEAN_EXEC_TIME_NS
    from concourse.bass_utils import run_bass_kernel_spmd

    gamma_np = np.asarray(gamma, dtype=np.float32)
    beta_np = np.asarray(beta, dtype=np.float32)
    apply_affine = not (np.all(gamma_np == 1.0) and np.all(beta_np == 0.0))

    nc = _get_program(apply_affine)
    in_maps = _prep_inputs(x, adj_bool, node_mask, W, a_l, a_r,
                           gamma_np, beta_np, apply_affine)
    trace = bool(int(os.environ.get("GAT_TRACE", "0")))
    res = run_bass_kernel_spmd(nc, in_maps, list(range(NCORES)), trace=trace)
    LAST_EXEC_TIME_NS = res.exec_time_ns
    LAST_MEAN_EXEC_TIME_NS = res.mean_exec_time_ns
    out = np.stack([np.asarray(r["out"], dtype=np.float32) for r in res.results])
    return out


# revision 15
# speedup vs baseline: 1.4204x; 1.4204x over previous
"""GAT layer (gnn_message_passing) Trainium2 Bass kernel, v4.

Per-core work (data-parallel over batch B=8, one graph per NeuronCore):
  h   = (x*m) @ W
  e   = leakyrelu(e_l[i] + e_r[j]),  e_l = h@a_l, e_r = h@a_r
  attn= softmax_j(adj&mask ? e : -inf)
  out = LN((attn @ h + x*m) * m) * gamma + beta

Softmax numerator (i-only factors cancel):
  q[j,i] = adj[i,j] * m_j * exp(lrelu(s) - 0.2 e_l[i]),  s = e_l[i]+e_r[j]
Two equivalent forms drive two production routes for the [128,2048] w
blocks ("v" on DVE, "s" on ScalarE + SDMA):
  v: q = adj * max(El8[i]*(Er8*r)[j], r[j]),  El8=exp(.8 e_l), r=m*exp(.2 e_r)
     -> tensor_scalar(mult+max vs el8 broadcast) + tensor_tensor(* adj01)
  s: q = exp(relu(0.8 e_l[i] + 0.8 e_r[j] - M_j) + 0.2 e_r[j] - M_j + A[i,j])
     -> ScalarE Relu(scale/bias) ; SWDGE dma ADD-accumulates A (fp8 {0,-1e4},
        cast in flight) ; ScalarE Exp(bias).  Zero DVE work.
The e_l broadcast row is built on-chip: row = wal^T @ xT (PE), replicated
by a ones outer-product (PE) -> exp on ScalarE.  No DRAM bounce.

Matmuls: w blocks STATIONARY, rhs = [ones | h | hs] (hs = sum_d h), so one
stream yields out[i,d], the softmax denominator and sum_d out.  Blocks are
processed v-routes first, s-routes last (PE FIFO never waits on the
slow-latency ScalarE pipeline).  LN: mean via the hs column, variance via
ScalarE Square(accum_out); node mask folds into the final affine.
"""

import os
import sys

import numpy as np

if "/opt/trn_rl_repo" not in sys.path:
    sys.path.insert(0, "/opt/trn_rl_repo")

B, N, D = 8, 2048, 128
NB = N // 128
ALPHA = 0.2
EPS = 1e-5
NCORES = 8
MASK_BIG = 10000.0

# Route assignment: blocks produced on ScalarE (+SDMA accumulate); rest DVE.
S_BLOCKS = (2, 5, 8, 11, 13, 15)

_PROG_CACHE = {}
RACE_DETECT = True
LAST_EXEC_TIME_NS = None
LAST_MEAN_EXEC_TIME_NS = None


def _patch_sem_clear():
    """This environment's walrus rejects EVENT_SEMAPHORE_RANGE_CLEAR
    ("ISA wrong length").  Tile's tail range-clear is unnecessary here
    (runtime resets between executions), so skip it."""
    import concourse.bass as bass

    if getattr(bass.BassEngine, "_gat_sem_clear_patched", False):
        return

    def sem_clear(self, sem):
        return None

    bass.BassEngine.sem_clear = sem_clear
    bass.BassEngine._gat_sem_clear_patched = True


def _split_waits(nc, mybir, max_waits=1):
    """This walrus build allows only one semaphore-wait slot per
    instruction; hoist extra waits onto standalone EventSemaphore
    carriers placed immediately before the offender on the same engine."""
    for f in nc.m.functions:
        for b in f.blocks:
            il = b.instructions
            k = 0
            while k < len(il):
                i = il[k]
                si = i.sync_info
                if si is not None and si.on_wait and len(si.on_wait) > max_waits:
                    waits = list(si.on_wait)
                    extra, keep = waits[:-max_waits], waits[-max_waits:]
                    for j, w in enumerate(extra):
                        ev = mybir.InstEventSemaphore(
                            name=f"{i.name}-wsplit{j}",
                            engine=i.engine,
                            debug=i.debug,
                            sync_info=mybir.SyncInfo(on_wait=[w], on_update=[]),
                        )
                        il.insert(k + j, ev)
                    k += len(extra)
                    i.sync_info = mybir.SyncInfo(
                        on_wait=keep, on_update=list(si.on_update or []))
                k += 1
    return nc


def _knobs():
    v = os.environ.get("GAT_S_BLOCKS")
    if v is not None:
        return tuple(int(x) for x in v.split(",") if x != "")
    return S_BLOCKS


def _build_program(apply_affine: bool):
    import concourse.bass as bass
    import concourse.tile as tile
    from concourse import mybir
    from concourse.masks import make_identity

    _patch_sem_clear()
    s_blocks = set(_knobs())
    v_blocks = [jb for jb in range(NB) if jb not in s_blocks]
    order = v_blocks + sorted(s_blocks)   # process v first, s last

    fp32 = mybir.dt.float32
    bf16 = mybir.dt.bfloat16
    fp8e5 = mybir.dt.float8e5
    A = mybir.AluOpType
    F = mybir.ActivationFunctionType

    nc = bass.Bass(use_seq_codegen=True, detect_race_conditions=RACE_DETECT)

    x_in = nc.declare_dram_parameter("x", [N, D], fp32, isOutput=False)
    xt_in = nc.declare_dram_parameter("xt", [D, N], bf16, isOutput=False)
    # adj8 rows: v-blocks hold {0,1}; s-blocks hold {0,-MASK_BIG}
    adj_in = nc.declare_dram_parameter("adj8", [N, N], fp8e5, isOutput=False)
    maskf = nc.declare_dram_parameter("maskf", [N], fp32, isOutput=False)
    # wfull = [W | W@1 | W@a_l | W@a_r | 1]
    wfull_in = nc.declare_dram_parameter("wfull", [D, D + 4], bf16, isOutput=False)
    if apply_affine:
        g_in = nc.declare_dram_parameter("gamma", [D], fp32, isOutput=False)
        b_in = nc.declare_dram_parameter("beta", [D], fp32, isOutput=False)
    out_d = nc.declare_dram_parameter("out", [N, D], fp32, isOutput=True)

    with tile.TileContext(nc) as tc:
        with tc.tile_pool(name="persist", bufs=1) as per:
            eps_col = per.tile([128, 1], fp32)
            nc.vector.memset(eps_col, EPS)
            warm = per.tile([128, 1], fp32)
            nc.scalar.activation(out=warm, in_=eps_col, func=F.Exp)
            ones_row = per.tile([1, 128], bf16)
            nc.vector.memset(ones_row, 1.0)

            m_col = per.tile([128, NB], fp32)
            nc.sync.dma_start(out=m_col, in_=maskf[:].rearrange("(b p) -> p b", p=128))
            w_full = per.tile([128, D + 4], bf16)
            nc.sync.dma_start(out=w_full, in_=wfull_in[:, :])
            xT_sb = per.tile([128, N], bf16)
            nc.sync.dma_start(out=xT_sb, in_=xt_in[:, :])
            if apply_affine:
                def bcast(ap, parts=128):
                    return bass.AP(tensor=ap.tensor, offset=ap.offset,
                                   ap=[[0, parts]] + list(ap.ap))
                g_bc = per.tile([128, D], fp32)
                nc.sync.dma_start(out=g_bc, in_=bcast(g_in[:]))
                b_bc = per.tile([128, D], fp32)
                nc.sync.dma_start(out=b_bc, in_=bcast(b_in[:]))

            # adjacency prefetch for v-blocks: fp8 {0,1} -> bf16 (SWDGE cast)
            adj_slot = {jb: k for k, jb in enumerate(v_blocks)}
            adj_all = per.tile([128, len(v_blocks), N], bf16)
            runs = []
            for jb in v_blocks:
                if runs and runs[-1][1] == jb:
                    runs[-1] = (runs[-1][0], jb + 1)
                else:
                    runs.append((jb, jb + 1))
            for a, b_ in runs:
                nc.gpsimd.dma_start(
                    out=adj_all[:, adj_slot[a]:adj_slot[a] + (b_ - a), :],
                    in_=adj_in[a * 128:b_ * 128, :].rearrange(
                        "(c p) i -> p c i", p=128))

            x_all = per.tile([128, NB, D], fp32)
            # hel per block: [ones | h(128) | hs | e_l | e_r | xs]
            hel_all = per.tile([128, NB, D + 6], bf16)
            nc.vector.memset(hel_all[:, :, 0], 1.0)
            el8_bc = per.tile([128, N], bf16)     # exp(0.8 e_l) broadcast
            elr_bc = per.tile([128, N], bf16)     # raw e_l broadcast
            er8r_col = per.tile([128, NB], fp32)
            r_col = per.tile([128, NB], fp32)
            b1_col = per.tile([128, NB], fp32)    # .8 e_r - BIG(1-m)
            b2_col = per.tile([128, NB], fp32)    # .2 e_r - BIG(1-m)
            z_all = per.tile([128, NB, D], fp32)
            z2s_col = per.tile([128, NB], fp32)
            rc_col = per.tile([128, NB], fp32)
            mu_col = per.tile([128, NB], fp32)
            rstdm_col = per.tile([128, NB], fp32)

            # ---- prep A: e_l row via PE, broadcast via PE outer ---------
            with tc.tile_pool(name="ppa_ps", bufs=1, space="PSUM") as ppa_ps:
                elrow_ps = ppa_ps.tile([1, N], fp32)
                for s in range(4):
                    nc.tensor.matmul(elrow_ps[:, s * 512:(s + 1) * 512],
                                     lhsT=w_full[:, D + 1:D + 2],
                                     rhs=xT_sb[:, s * 512:(s + 1) * 512],
                                     start=True, stop=True)
                elrow_sb = per.tile([1, N], bf16)
                nc.vector.tensor_copy(out=elrow_sb, in_=elrow_ps)
            with tc.tile_pool(name="ppb_ps", bufs=1, space="PSUM") as ppb_ps:
                elbc_ps = ppb_ps.tile([128, N], fp32)
                for s in range(4):
                    nc.tensor.matmul(elbc_ps[:, s * 512:(s + 1) * 512],
                                     lhsT=ones_row,
                                     rhs=elrow_sb[:, s * 512:(s + 1) * 512],
                                     start=True, stop=True)
                nc.scalar.activation(out=el8_bc, in_=elbc_ps, func=F.Exp,
                                     scale=0.8)
                nc.vector.tensor_copy(out=elr_bc, in_=elbc_ps)

            # ---- prep B: h|hs|el|er|xs blocks, e_r exps -----------------
            with (
                tc.tile_pool(name="pp", bufs=3) as pp,
                tc.tile_pool(name="pp_ps", bufs=2, space="PSUM") as pp_ps,
            ):
                for ib in range(NB):
                    hr_ps = pp_ps.tile([128, D + 4], fp32, tag="hr")
                    nc.tensor.matmul(hr_ps,
                                     lhsT=xT_sb[:, ib * 128:(ib + 1) * 128],
                                     rhs=w_full, start=True, stop=True)
                    if ib % 2 == 0:
                        nc.scalar.activation(out=hel_all[:, ib, 1:D + 5],
                                             in_=hr_ps, func=F.Copy)
                    else:
                        nc.vector.tensor_copy(out=hel_all[:, ib, 1:D + 5],
                                              in_=hr_ps)
                    nc.sync.dma_start(out=x_all[:, ib, :],
                                      in_=x_in[ib * 128:(ib + 1) * 128, :])

                er_v = hel_all[:, :, D + 3]
                er8_col = pp.tile([128, NB], fp32, tag="er8")
                nc.scalar.activation(out=er8_col, in_=er_v, func=F.Exp, scale=0.8)
                er2_col = pp.tile([128, NB], fp32, tag="er2")
                nc.scalar.activation(out=er2_col, in_=er_v, func=F.Exp, scale=0.2)
                nc.vector.tensor_tensor(out=r_col, in0=er2_col, in1=m_col,
                                        op=A.mult)
                nc.vector.tensor_tensor(out=er8r_col, in0=er8_col, in1=r_col,
                                        op=A.mult)
                # s-route biases: bk = k*e_r + BIG*(m-1)
                mb_col = pp.tile([128, NB], fp32, tag="mb")
                nc.vector.tensor_scalar(out=mb_col, in0=m_col,
                                        scalar1=MASK_BIG, scalar2=-MASK_BIG,
                                        op0=A.mult, op1=A.add)
                nc.vector.tensor_scalar(out=b1_col, in0=er_v, scalar1=0.8,
                                        scalar2=None, op0=A.mult)
                nc.vector.tensor_tensor(out=b1_col, in0=b1_col, in1=mb_col,
                                        op=A.add)
                nc.vector.tensor_scalar(out=b2_col, in0=er_v, scalar1=0.2,
                                        scalar2=None, op0=A.mult)
                nc.vector.tensor_tensor(out=b2_col, in0=b2_col, in1=mb_col,
                                        op=A.add)

            # ---- main: per-block w production + matmuls -----------------
            with (
                tc.tile_pool(name="mm_ps", bufs=1, space="PSUM") as mm_ps_pool,
                tc.tile_pool(name="wp", bufs=4) as wp,
            ):
                out_all = mm_ps_pool.tile([128, NB, 256], fp32)
                # s-route stage 1 for ALL s-blocks up front: ScalarE relu then
                # SWDGE add-accumulate of the {0,-BIG} adjacency (cast fp8->bf16).
                # Keeps the ScalarE FIFO free of DMA-wait stalls.
                su = {}
                for jb in sorted(s_blocks):
                    u_t = per.tile([128, N], bf16, name=f"su{jb}", tag=f"su{jb}")
                    nc.scalar.activation(out=u_t, in_=elr_bc, func=F.Relu,
                                         scale=0.8, bias=b1_col[:, jb:jb + 1])
                    nc.gpsimd.dma_start(
                        out=u_t, in_=adj_in[jb * 128:(jb + 1) * 128, :],
                        accum_op=A.add)
                    su[jb] = u_t
                for k, jb in enumerate(order):
                    if jb in s_blocks:
                        w_t = su[jb]
                        nc.scalar.activation(out=w_t, in_=w_t, func=F.Exp,
                                             bias=b2_col[:, jb:jb + 1])
                    else:
                        w_t = wp.tile([128, N], bf16, tag="w")
                        nc.vector.tensor_scalar(
                            out=w_t, in0=el8_bc,
                            scalar1=er8r_col[:, jb:jb + 1],
                            scalar2=r_col[:, jb:jb + 1],
                            op0=A.mult, op1=A.max)
                        nc.vector.tensor_tensor(
                            out=w_t, in0=w_t,
                            in1=adj_all[:, adj_slot[jb], :], op=A.mult)
                    st, sp = (k == 0), (k == NB - 1)
                    for ib in range(NB):
                        nc.tensor.matmul(
                            out_all[:, ib, 0:130],
                            lhsT=w_t[:, ib * 128:(ib + 1) * 128],
                            rhs=hel_all[:, jb, 0:130],
                            start=st, stop=sp)

                # ---- epilogue -------------------------------------------
                with tc.tile_pool(name="ep", bufs=4) as ep:
                    rs_sb = ep.tile([128, NB], fp32, tag="rs")
                    nc.vector.tensor_copy(out=rs_sb, in_=out_all[:, :, 0])
                    nc.vector.reciprocal(out=rc_col, in_=rs_sb)

                    # mu = (rc*S1 + xs)/D
                    s1_sb = ep.tile([128, NB], fp32, tag="s1")
                    nc.vector.tensor_copy(out=s1_sb, in_=out_all[:, :, 129])
                    nc.vector.tensor_tensor(out=mu_col, in0=s1_sb, in1=rc_col,
                                            op=A.mult)
                    xs_v = hel_all[:, :, D + 4]
                    nc.vector.tensor_tensor(out=mu_col, in0=mu_col, in1=xs_v,
                                            op=A.add)
                    nc.vector.tensor_scalar(out=mu_col, in0=mu_col,
                                            scalar1=1.0 / D, scalar2=None,
                                            op0=A.mult)

                    for ib in range(NB):
                        z1 = ep.tile([128, 128], fp32, tag="z1")
                        nc.vector.tensor_scalar(out=z1,
                                                in0=out_all[:, ib, 1:129],
                                                scalar1=rc_col[:, ib:ib + 1],
                                                scalar2=None, op0=A.mult)
                        zeng = nc.gpsimd if ib % 2 else nc.vector
                        zeng.tensor_tensor(
                            out=z_all[:, ib, :], in0=z1, in1=x_all[:, ib, :],
                            op=A.add)
                        sq = ep.tile([128, 128], fp32, tag="sq")
                        nc.scalar.activation(out=sq, in_=z_all[:, ib, :],
                                             func=F.Square,
                                             accum_out=z2s_col[:, ib:ib + 1])

                    mu2_col = ep.tile([128, NB], fp32, tag="mu2")
                    nc.vector.tensor_tensor(out=mu2_col, in0=mu_col, in1=mu_col,
                                            op=A.mult)
                    var_col = ep.tile([128, NB], fp32, tag="var")
                    nc.vector.tensor_scalar(out=var_col, in0=z2s_col,
                                            scalar1=1.0 / D, scalar2=None,
                                            op0=A.mult)
                    nc.vector.tensor_tensor(out=var_col, in0=var_col,
                                            in1=mu2_col, op=A.subtract)
                    lnv_col = ep.tile([128, NB], fp32, tag="lnv")
                    nc.scalar.activation(out=lnv_col, in_=var_col, func=F.Ln,
                                         bias=eps_col, scale=1.0)
                    rstd_col = ep.tile([128, NB], fp32, tag="rstd")
                    nc.scalar.activation(out=rstd_col, in_=lnv_col, func=F.Exp,
                                         scale=-0.5)
                    nc.vector.tensor_tensor(out=rstdm_col, in0=rstd_col,
                                            in1=m_col, op=A.mult)

                    for ib in range(NB):
                        o_t = ep.tile([128, D], fp32, tag="o")
                        nc.vector.tensor_scalar(
                            out=o_t, in0=z_all[:, ib, :],
                            scalar1=mu_col[:, ib:ib + 1],
                            scalar2=rstdm_col[:, ib:ib + 1],
                            op0=A.subtract, op1=A.mult)
                        if apply_affine:
                            nc.vector.tensor_tensor(out=o_t, in0=o_t, in1=g_bc,
                                                    op=A.mult)
                            nc.vector.tensor_tensor(out=o_t, in0=o_t, in1=b_bc,
                                                    op=A.add)
                        nc.sync.dma_start(out=out_d[ib * 128:(ib + 1) * 128, :],
                                          in_=o_t)
    return _split_waits(nc, mybir)


def _get_program(apply_affine: bool):
    key = (apply_affine, _knobs())
    if key not in _PROG_CACHE:
        _PROG_CACHE[key] = _build_program(apply_affine)
    return _PROG_CACHE[key]


def _prep_inputs(x, adj_bool, node_mask, W, a_l, a_r, gamma, beta, apply_affine):
    import ml_dtypes

    bf16 = ml_dtypes.bfloat16
    f8e5 = ml_dtypes.float8_e5m2
    s_blocks = set(_knobs())
    x = np.asarray(x, dtype=np.float32)
    adj_bool = np.asarray(adj_bool)
    node_mask = np.asarray(node_mask)
    W32 = np.asarray(W, dtype=np.float32)
    wal = W32 @ np.asarray(a_l, np.float32)
    war = W32 @ np.asarray(a_r, np.float32)
    ws = W32.sum(axis=1)
    ones = np.ones((D, 1), np.float32)
    wfull = np.concatenate([W32, ws[:, None], wal[:, None], war[:, None], ones],
                           axis=1)
    wfull_bf = np.ascontiguousarray(wfull.astype(bf16))
    # per-row-block adjacency encoding: v-blocks {0,1}; s-blocks
    # {-BIG for missing edge, 0 for edge} (added to relu output pre-exp)
    s_sel = np.zeros((N, 1), np.float32)
    for jb in s_blocks:
        s_sel[jb * 128:(jb + 1) * 128] = 1.0
    in_maps = []
    for b in range(NCORES):
        adjT = adj_bool[b].T.astype(np.float32)
        adj8 = (adjT * (1.0 - s_sel) + (adjT - 1.0) * MASK_BIG * s_sel
                ).astype(f8e5)
        m = {
            "x": np.ascontiguousarray(x[b]),
            "xt": np.ascontiguousarray(x[b].T.astype(bf16)),
            "adj8": np.ascontiguousarray(adj8),
            "maskf": np.ascontiguousarray(node_mask[b].astype(np.float32)),
            "wfull": wfull_bf,
        }
        if apply_affine:
            m["gamma"] = np.ascontiguousarray(np.asarray(gamma, np.float32))
            m["beta"] = np.ascontiguousarray(np.asarray(beta, np.float32))
        in_maps.append(m)
    return in_maps


def kernel(x, adj_bool, node_mask, W, a_l, a_r, gamma, beta):
    global LAST_EXEC_TIME_NS, LAST_MEAN_EXEC_TIME_NS
    from concourse.bass_utils import run_bass_kernel_spmd

    gamma_np = np.asarray(gamma, dtype=np.float32)
    beta_np = np.asarray(beta, dtype=np.float32)
    apply_affine = not (np.all(gamma_np == 1.0) and np.all(beta_np == 0.0))

    nc = _get_program(apply_affine)
    in_maps = _prep_inputs(x, adj_bool, node_mask, W, a_l, a_r,
                           gamma_np, beta_np, apply_affine)
    trace = bool(int(os.environ.get("GAT_TRACE", "0")))
    res = run_bass_kernel_spmd(nc, in_maps, list(range(NCORES)), trace=trace)
    LAST_EXEC_TIME_NS = res.exec_time_ns
    LAST_MEAN_EXEC_TIME_NS = res.mean_exec_time_ns
    out = np.stack([np.asarray(r["out"], dtype=np.float32) for r in res.results])
    return out
